# revision 13
# baseline (speedup 1.0000x reference)
"""Expert-choice MoE kernel for Trainium2 (8 NeuronCores, expert-parallel).

Strategy (one expert per core, expert e == core id):
  - host pre-transposes: x^T token-slice per core (for the f32 gate matmul),
    W1[e]^T / W3[e]^T / W2[e]^T in bf16, Wg^T replicated.
  - prewarm: a tiny AllToAll issued first thing absorbs the CC-ring
    first-call setup latency while the gate phase runs.
  - gate: each core computes f32 scores [E, N/8] for its token slice;
    AllToAll swaps shards so core e ends with scores[e, :] for ALL tokens.
  - top-k (capacity = N/E): exact threshold via 8 rounds of 4-bit radix
    bisection on sortable-uint32 keys; per-round counts are reduced across
    partitions with PE matmuls (ones-vector contraction) instead of gpsimd,
    keeping the whole loop on DVE+PE with low latency.
  - compaction: gpsimd sparse_gather compresses selected token ids (critical
    path) and softmax numerators (relaxed path) separately, so the token
    gather starts ~8us earlier.
  - dispatch: indirect-DMA row gather of selected tokens, cast bf16,
    PE-transpose to [C, cap] layout.
  - SwiGLU: h1/h3 = W1/W3 @ xg^T (PSUM), a = silu(h1)*h3 (bf16),
    y = a^T @ W2^T, scaled by softmax weights.  W1/W3/W2 are fully
    preloaded into SBUF during the routing phase.
  - combine: each core returns compact [cap, C] rows + routing metadata;
    host does the (unique-index) scatter-add across experts.
"""

import numpy as np
import ml_dtypes

import concourse.bass as bass
import concourse.bacc as bacc
import concourse.mybir as mybir
import concourse.tile as tile
from concourse import bass_isa
from concourse import bass_utils
from concourse import library_config


FP = mybir.dt.float32
BF = mybir.dt.bfloat16
U32 = mybir.dt.uint32
I32 = mybir.dt.int32

NCORES = 8


class Cfg:
    def __init__(self, N=8192, C=1024, H=2048, E=8):
        assert N % (16 * 128) == 0 and C % 128 == 0 and H % 128 == 0
        self.N, self.C, self.H, self.E = N, C, H, E
        self.CAP = N // E                      # tokens kept per expert
        self.TS = N // NCORES                  # tokens gated per core
        self.CK = C // 128                     # C chunks (contraction)
        self.HK = H // 128                     # H chunks
        self.TOKCH = min(256, self.CAP)        # token free-dim chunk for h/a
        self.NTOK = self.CAP // self.TOKCH     # number of token chunks
        self.G = self.CAP // 128               # token groups of 128 (gather/y)
        self.GPT = self.TOKCH // 128           # groups per token chunk
        self.GTS = min(512, self.TS)           # gate token chunk
        assert self.CAP % 128 == 0 and self.TS % self.GTS == 0


def ts(i, n):
    return slice(i * n, (i + 1) * n)


def build_program(cfg: Cfg):
    """Build the single SPMD Bass program (same for all 8 cores)."""
    N, C, H, E = cfg.N, cfg.C, cfg.H, cfg.E
    CAP = cfg.CAP

    nc = bacc.Bacc("TRN2", target_bir_lowering=False, debug=False,
                   enable_asserts=False, num_devices=NCORES)

    # --- I/O ---
    xbf_d = nc.dram_tensor("xbf", [N, C], BF, kind="ExternalInput")
    xts_d = nc.dram_tensor("xts", [C, cfg.TS], FP, kind="ExternalInput")
    # gate weight pre-shuffled on host: [p, k, e] = Wg[e, k*128+p]
    wgt_d = nc.dram_tensor("wgT", [128, C // 128, E], FP, kind="ExternalInput")
    # W1/W3 transposed and pre-tiled: [p, (hc ck f)] = W[e].T block layout
    w1t_d = nc.dram_tensor("w1T", [128, cfg.HK * cfg.CK * 128], BF,
                           kind="ExternalInput")
    w3t_d = nc.dram_tensor("w3T", [128, cfg.HK * cfg.CK * 128], BF,
                           kind="ExternalInput")
    w2t_d = nc.dram_tensor("w2T", [H, C], BF, kind="ExternalInput")

    # host-provided constants
    identbf_d = nc.dram_tensor("identbf", [128, 128], BF, kind="ExternalInput")
    identf_d = nc.dram_tensor("identf", [16, 16], FP, kind="ExternalInput")
    # iotash[p, r*15 + (j-1)] = float(j << (12 - 4r)), j in 1..15
    iotash_d = nc.dram_tensor("iotash", [128, 60], FP, kind="ExternalInput")
    idsf_d = nc.dram_tensor("idsf", [16, N // 16], FP, kind="ExternalInput")

    y_d = nc.dram_tensor("y", [CAP, C], FP, kind="ExternalOutput")
    # meta[r, c] = token id (f32) of compact slot r*16+c
    meta_d = nc.dram_tensor("meta", [CAP // 16, 16], FP, kind="ExternalOutput")
    dbg_d = nc.dram_tensor("dbg", [1, 4], FP, kind="ExternalOutput")

    SG_F = N // 16          # sparse-gather input free size (<= 512)
    SG_O = CAP // 16        # sparse-gather output free size (<= 512)
    S128F = N // 128        # [128, S128F] score layout for bisection
    assert SG_F <= 512 and SG_O <= 512

    from contextlib import ExitStack
    with tile.TileContext(nc) as tc, ExitStack() as es:
        cpool = es.enter_context(tc.tile_pool(name="const", bufs=1))
        wpool = es.enter_context(tc.tile_pool(name="weights", bufs=1))
        gatep = es.enter_context(tc.tile_pool(name="gate", bufs=1))
        rpool = es.enter_context(tc.tile_pool(name="route", bufs=1))
        gpool = es.enter_context(tc.tile_pool(name="gath", bufs=2))
        apool = es.enter_context(tc.tile_pool(name="acts", bufs=1))
        ypool = es.enter_context(tc.tile_pool(name="yout", bufs=2))
        dram = es.enter_context(tc.tile_pool(name="dram", bufs=1, space="DRAM"))
        ps_x = es.enter_context(tc.tile_pool(name="ps_x", bufs=2, space="PSUM"))
        ps_h1 = es.enter_context(tc.tile_pool(name="ps_h1", bufs=2, space="PSUM"))
        ps_h3 = es.enter_context(tc.tile_pool(name="ps_h3", bufs=2, space="PSUM"))
        ps_y = es.enter_context(tc.tile_pool(name="ps_y", bufs=2, space="PSUM"))

        # --- prewarm collective: absorb CC first-call setup during the gate
        pw_sb = cpool.tile([8, 16], FP, tag="pw_sb")
        nc.vector.memset(pw_sb[:], 0.0)
        pw_in = dram.tile([8, 16], FP, tag="pw_in")
        pw_out = dram.tile([8, 16], FP, tag="pw_out")
        nc.sync.dma_start(pw_in[:], pw_sb[:])
        nc.gpsimd.collective_compute(
            "AllToAll", mybir.AluOpType.bypass,
            replica_groups=[list(range(NCORES))],
            ins=[pw_in[:].opt()], outs=[pw_out[:].opt()])

        # --- constants ---
        ident_bf = cpool.tile([128, 128], BF, tag="ident_bf")
        nc.sync.dma_start(ident_bf[:], identbf_d[:, :])
        ident_f = cpool.tile([16, 16], FP, tag="ident_f")
        nc.sync.dma_start(ident_f[:], identf_d[:, :])
        iotash = cpool.tile([128, 60], FP, tag="iotash")
        nc.sync.dma_start(iotash[:], iotash_d[:, :])
        ids_f = cpool.tile([16, SG_F], FP, tag="ids_f")
        nc.sync.dma_start(ids_f[:], idsf_d[:, :])
        neg1 = cpool.tile([16, SG_F], FP, tag="neg1")
        nc.vector.memset(neg1[:], -1.0)
        ones_row = cpool.tile([1, 128], FP, tag="ones_row")
        nc.vector.memset(ones_row[:], 1.0)
        ones_col = cpool.tile([128, 1], FP, tag="ones_col")
        nc.vector.memset(ones_col[:], 1.0)
        zero128 = cpool.tile([128, S128F], FP, tag="zero128")
        nc.vector.memset(zero128[:], 0.0)

        # --- PE p-state warmup: ~3us of junk matmuls (memset operands, one
        # PSUM accumulation chain -> no DMA dependency, no inter-op sems) so
        # the gate matmuls run at full clock instead of the 0.65/1.2 GHz
        # cold tiers.
        junk = cpool.tile([128, 512], BF, tag="junk")
        nc.vector.memset(junk[:], 0.0)
        wrm = ps_x.tile([128, 512], FP, tag="xpose_ps", name="warm_ps")
        for i in range(8):
            nc.tensor.matmul(wrm[:], lhsT=junk[:, :128], rhs=junk[:],
                             start=(i == 0), stop=(i == 7))

        # --- gate: scores[E, TS] for this core's token slice (f32) ---
        wg_sb = gatep.tile([128, cfg.CK, E], FP, tag="wg")
        nc.sync.dma_start(wg_sb[:], wgt_d[:, :, :])
        scores_sb = gatep.tile([E, cfg.TS], FP, tag="scores")
        ngt = cfg.TS // cfg.GTS
        gps = []
        for t in range(ngt):
            gp = ps_y.tile([E, cfg.GTS], FP, tag="ypsum", name=f"gate_ps_{t}")
            gps.append(gp)
        xt_last = None
        for k in range(cfg.CK):
            xt = gatep.tile([128, cfg.TS], FP, tag="xts", bufs=2,
                            name=f"xts_{k}")
            nc.sync.dma_start(xt[:], xts_d[ts(k, 128), :])
            xt_last = xt
            for t in range(ngt):
                nc.tensor.matmul(gps[t][:], lhsT=wg_sb[:, k, :],
                                 rhs=xt[:, ts(t, cfg.GTS)],
                                 start=(k == 0), stop=(k == cfg.CK - 1))
        for t in range(ngt):
            nc.vector.tensor_copy(scores_sb[:, ts(t, cfg.GTS)], gps[t][:])

        # --- AllToAll: shard j of core c (expert-j scores of slice c) -> core j
        cc_in = dram.tile([E, cfg.TS], FP, tag="cc_in")
        cc_out = dram.tile([E, cfg.TS], FP, tag="cc_out")
        nc.sync.dma_start(cc_in[:], scores_sb[:])
        nc.gpsimd.collective_compute(
            "AllToAll", mybir.AluOpType.bypass,
            replica_groups=[list(range(NCORES))],
            ins=[cc_in[:].opt()], outs=[cc_out[:].opt()])

        # --- load my-expert scores in two layouts ---
        s128 = rpool.tile([128, S128F], FP, tag="s128")
        nc.sync.dma_start(
            s128[:], cc_out[:].rearrange("a (p f) -> (a p) f", p=128 // E))
        s16 = rpool.tile([16, SG_F], FP, tag="s16")
        nc.sync.dma_start(
            s16[:], cc_out[:].rearrange("a (p f) -> (a p) f", p=16 // E))

        # --- resident weights on the qAct HWDGE ring, fenced behind the
        # A2A output: the CC stack's setup/ops measurably slow down (~20us)
        # when bulk DMA streams concurrently, so keep HBM quiet until the
        # exchange is done.  12.6MB land ~40us later, still ~35us before
        # the FFN first needs W1.
        fence_d = dram.tile([1, 1], FP, tag="fence_d")
        nc.scalar.dma_start(fence_d[:], s128[:1, :1])
        w1sb = wpool.tile([128, cfg.HK * cfg.CK * 128], BF, tag="w1sb")
        nc.scalar.dma_start(w1sb[:], w1t_d[:, :])
        w3sb = wpool.tile([128, cfg.HK * cfg.CK * 128], BF, tag="w3sb")
        nc.scalar.dma_start(w3sb[:], w3t_d[:, :])
        w2all = wpool.tile([128, cfg.HK, C], BF, tag="w2all")
        nc.scalar.dma_start(w2all[:],
                            w2t_d[:, :].rearrange("(k p) c -> p k c", p=128))
        w2sb = [w2all[:, k, :] for k in range(cfg.HK)]

        # --- softmax max via PE transpose (no gpsimd) ---
        mx = rpool.tile([16, 1], FP, tag="mx")
        nc.vector.reduce_max(mx[:], s16[:], axis=mybir.AxisListType.X)
        mxt = ps_x.tile([1, 16], FP, tag="xpose_ps", name="mx_t")
        nc.tensor.transpose(mxt[:], mx[:], ident_f[:])
        mxs = rpool.tile([1, 1], FP, tag="mxs")
        nc.vector.reduce_max(mxs[:], mxt[:], axis=mybir.AxisListType.X)
        mxp = ps_x.tile([16, 1], FP, tag="xpose_ps", name="mx_bc")
        nc.tensor.matmul(mxp[:], lhsT=ones_row[:, :16], rhs=mxs[:],
                         start=True, stop=True)
        negm = rpool.tile([16, 1], FP, tag="negm")
        nc.vector.tensor_scalar(negm[:], mxp[:], -1.0, scalar2=None,
                                op0=mybir.AluOpType.mult)
        # exp(s - max) for all tokens; independent of the threshold so it
        # overlaps the bisection below.
        expm = rpool.tile([16, SG_F], FP, tag="expm")
        nc.scalar.activation(expm[:], s16[:],
                             mybir.ActivationFunctionType.Exp, bias=negm[:])

        # --- sortable uint32 keys on [128, S128F] ---
        kb = s128[:].bitcast(U32)
        sgn = rpool.tile([128, S128F], U32, tag="sgn")
        nc.vector.tensor_scalar(sgn[:], kb, 31, scalar2=None,
                                op0=mybir.AluOpType.logical_shift_right)
        tneg = rpool.tile([128, S128F], U32, tag="tneg")
        nc.vector.tensor_scalar(tneg[:], kb, 0xFFFFFFFF, scalar2=None,
                                op0=mybir.AluOpType.bitwise_xor)
        tpos = rpool.tile([128, S128F], U32, tag="tpos")
        nc.vector.tensor_scalar(tpos[:], kb, 0x80000000, scalar2=None,
                                op0=mybir.AluOpType.bitwise_or)
        keys = rpool.tile([128, S128F], U32, tag="keys")
        nc.vector.select(keys[:], sgn[:], tneg[:], tpos[:])

        # hi/lo 16-bit halves as exact f32 values
        khi_u = rpool.tile([128, S128F], U32, tag="khi_u")
        nc.vector.tensor_scalar(khi_u[:], keys[:], 16, scalar2=None,
                                op0=mybir.AluOpType.logical_shift_right)
        klo_u = rpool.tile([128, S128F], U32, tag="klo_u")
        nc.vector.tensor_scalar(klo_u[:], keys[:], 0xFFFF, scalar2=None,
                                op0=mybir.AluOpType.bitwise_and)
        khi = rpool.tile([128, S128F], FP, tag="khi")
        nc.vector.tensor_copy(khi[:], khi_u[:])
        klo = rpool.tile([128, S128F], FP, tag="klo")
        nc.vector.tensor_copy(klo[:], klo_u[:])

        # --- exact threshold via two-phase 16-bit radix bisection, counts
        # reduced across partitions on the PE (ones-vector contraction).
        def bisect16(vals, capv, tagp):
            """Max lo16 (f32 scalar + [128,1] broadcast) with
            count(vals >= lo16) >= cap; vals f32 ints < 2^16."""
            lo_sc = rpool.tile([1, 1], FP, tag=f"{tagp}_lo")
            nc.vector.memset(lo_sc[:], 0.0)
            lo_bc = rpool.tile([128, 1], FP, tag=f"{tagp}_lobc")
            nc.vector.memset(lo_bc[:], 0.0)
            for r in range(4):
                sh = 12 - 4 * r
                cand = rpool.tile([128, 15], FP, tag=f"{tagp}_cand",
                                  name=f"{tagp}_cand_{r}")
                nc.vector.tensor_tensor(
                    out=cand[:], in0=iotash[:, ts(r, 15)],
                    in1=lo_bc[:].to_broadcast([128, 15]),
                    op=mybir.AluOpType.add)
                gef = rpool.tile([128, 15, S128F], FP, tag=f"{tagp}_gef")
                nc.vector.tensor_tensor(
                    out=gef[:],
                    in0=vals[:].unsqueeze(1).to_broadcast([128, 15, S128F]),
                    in1=cand[:].unsqueeze(2).to_broadcast([128, 15, S128F]),
                    op=mybir.AluOpType.is_ge)
                cnt = rpool.tile([128, 15], FP, tag=f"{tagp}_cnt")
                nc.vector.reduce_sum(cnt[:], gef[:],
                                     axis=mybir.AxisListType.X)
                totp = ps_x.tile([1, 15], FP, tag="xpose_ps",
                                 name=f"{tagp}_tot_{r}")
                nc.tensor.matmul(totp[:], lhsT=ones_col[:], rhs=cnt[:],
                                 start=True, stop=True)
                gek = rpool.tile([1, 15], FP, tag=f"{tagp}_gek")
                nc.vector.tensor_tensor(
                    out=gek[:], in0=totp[:],
                    in1=capv[:].to_broadcast([1, 15]),
                    op=mybir.AluOpType.is_ge)
                nst = rpool.tile([1, 1], FP, tag=f"{tagp}_nst")
                nc.vector.reduce_sum(nst[:], gek[:], axis=mybir.AxisListType.X)
                stp = rpool.tile([1, 1], FP, tag=f"{tagp}_stp")
                nc.vector.tensor_scalar(stp[:], nst[:], float(1 << sh),
                                        scalar2=None,
                                        op0=mybir.AluOpType.mult)
                nc.vector.tensor_tensor(out=lo_sc[:], in0=lo_sc[:],
                                        in1=stp[:], op=mybir.AluOpType.add)
                # broadcast partition 0 -> 128 on the (idle) gpsimd engine:
                # ~0.4us vs ~1.2us for the fp32 ones-vector matmul + copy.
                nc.gpsimd.partition_broadcast(lo_bc[:], lo_sc[:])
            return lo_sc, lo_bc

        cap_t = rpool.tile([1, 1], FP, tag="cap_t")
        nc.vector.memset(cap_t[:], float(CAP))
        hi_sc, hi_bc = bisect16(khi, cap_t, "p1")
        # cap2 = CAP - count(khi > hi_star)
        gt = rpool.tile([128, S128F], FP, tag="gt")
        nc.vector.tensor_tensor(out=gt[:], in0=khi[:],
                                in1=hi_bc[:].to_broadcast([128, S128F]),
                                op=mybir.AluOpType.is_gt)
        gtc = rpool.tile([128, 1], FP, tag="gtc")
        nc.vector.reduce_sum(gtc[:], gt[:], axis=mybir.AxisListType.X)
        gtp = ps_x.tile([1, 1], FP, tag="xpose_ps", name="gt_tot")
        nc.tensor.matmul(gtp[:], lhsT=ones_col[:], rhs=gtc[:],
                         start=True, stop=True)
        cap2 = rpool.tile([1, 1], FP, tag="cap2")
        nc.vector.tensor_scalar(cap2[:], gtp[:], -1.0, scalar2=float(CAP),
                                op0=mybir.AluOpType.mult,
                                op1=mybir.AluOpType.add)
        # klo_eff = (khi == hi_star) ? klo : 0   (cand >= 1 so 0 never counts)
        eqm = rpool.tile([128, S128F], U32, tag="eqm")
        nc.vector.tensor_tensor(out=eqm[:], in0=khi[:],
                                in1=hi_bc[:].to_broadcast([128, S128F]),
                                op=mybir.AluOpType.is_equal)
        klo_eff = rpool.tile([128, S128F], FP, tag="klo_eff")
        nc.vector.select(klo_eff[:], eqm[:], klo[:], zero128[:])
        lo_sc, lo_bc = bisect16(klo_eff, cap2, "p2")

        # --- threshold back to float bits (on [128,1] then use row 0..15) ---
        hi_u = rpool.tile([128, 1], U32, tag="hi_u")
        nc.vector.tensor_copy(hi_u[:], hi_bc[:])
        lo_u = rpool.tile([128, 1], U32, tag="lo_u")
        nc.vector.tensor_copy(lo_u[:], lo_bc[:])
        key_u = rpool.tile([128, 1], U32, tag="key_u")
        nc.vector.tensor_scalar(key_u[:], hi_u[:], 16, scalar2=None,
                                op0=mybir.AluOpType.logical_shift_left)
        nc.vector.tensor_tensor(out=key_u[:], in0=key_u[:], in1=lo_u[:],
                                op=mybir.AluOpType.bitwise_or)
        sgn2 = rpool.tile([128, 1], U32, tag="sgn2")
        nc.vector.tensor_scalar(sgn2[:], key_u[:], 31, scalar2=None,
                                op0=mybir.AluOpType.logical_shift_right)
        tp2 = rpool.tile([128, 1], U32, tag="tp2")
        nc.vector.tensor_scalar(tp2[:], key_u[:], 0x80000000, scalar2=None,
                                op0=mybir.AluOpType.bitwise_xor)
        tn2 = rpool.tile([128, 1], U32, tag="tn2")
        nc.vector.tensor_scalar(tn2[:], key_u[:], 0xFFFFFFFF, scalar2=None,
                                op0=mybir.AluOpType.bitwise_xor)
        tbits = rpool.tile([128, 1], U32, tag="tbits")
        nc.vector.select(tbits[:], sgn2[:], tp2[:], tn2[:])
        thr_f = tbits[:].bitcast(FP)

        # --- selection mask in [16, SG_F] token layout ---
        mask16 = rpool.tile([16, SG_F], U32, tag="mask16")
        nc.vector.tensor_tensor(out=mask16[:], in0=s16[:],
                                in1=thr_f[:16, :].to_broadcast([16, SG_F]),
                                op=mybir.AluOpType.is_ge)
        # ids first: the token gather depends only on these
        sel_ids = rpool.tile([16, SG_F], FP, tag="sel_ids")
        nc.vector.select(sel_ids[:], mask16[:], ids_f[:], neg1[:])
        packed_i = rpool.tile([16, SG_O], FP, tag="packed_i")
        nf = rpool.tile([1, 1], U32, tag="nf")
        nc.gpsimd.sparse_gather(out=packed_i[:], in_=sel_ids[:],
                                num_found=nf[:])
        mps = ps_x.tile([SG_O, 16], FP, tag="xpose_ps", name="meta_t")
        nc.tensor.transpose(mps[:], packed_i[:], ident_f[:])
        meta_sb = rpool.tile([SG_O, 16], FP, tag="meta_sb")
        nc.vector.tensor_copy(meta_sb[:], mps[:])
        meta_bnc = dram.tile([SG_O, 16], FP, tag="meta_bnc")
        nc.sync.dma_start(meta_bnc[:], meta_sb[:])
        nc.sync.dma_start(meta_d[:, :], meta_sb[:])
        meta_lin = meta_bnc[:].rearrange("p f -> (p f)").unsqueeze(1)

        idx_cols = []
        for g in range(cfg.G):
            idf = rpool.tile([128, 1], FP, tag=f"idf_{g}")
            nc.sync.dma_start(idf[:], meta_lin[ts(g, 128)])
            idi = rpool.tile([128, 1], I32, tag=f"idi_{g}")
            nc.vector.tensor_copy(idi[:], idf[:])
            idx_cols.append(idi)

        # --- gather selected tokens, cast bf16, PE-transpose to [C, cap] ---
        xgT = [[None] * cfg.NTOK for _ in range(cfg.CK)]
        for c in range(cfg.CK):
            for t in range(cfg.NTOK):
                xgT[c][t] = apool.tile([128, cfg.TOKCH], BF,
                                       tag=f"xgt_{c}_{t}",
                                       name=f"xgt_{c}_{t}")
        for g in range(cfg.G):
            xg = gpool.tile([128, C], BF, tag="xg", bufs=3)
            nc.gpsimd.indirect_dma_start(
                out=xg[:], out_offset=None, in_=xbf_d[:, :],
                in_offset=bass.IndirectOffsetOnAxis(ap=idx_cols[g][:, :1],
                                                    axis=0))
            t, col = g // cfg.GPT, g % cfg.GPT
            for c in range(cfg.CK):
                tp = ps_x.tile([128, 128], BF, tag="xpose_ps")
                nc.tensor.transpose(tp[:], xg[:, ts(c, 128)], ident_bf[:])
                nc.vector.tensor_copy(xgT[c][t][:, ts(col, 128)], tp[:])

        # --- softmax weights path (relaxed: only needed by the y phase;
        # its sparse_gather queues on gpsimd after the gather desc-gens) ---
        expsel = rpool.tile([16, SG_F], FP, tag="expsel")
        nc.vector.tensor_tensor(out=expsel[:], in0=expm[:], in1=mask16[:],
                                op=mybir.AluOpType.mult)
        sel_w = rpool.tile([16, SG_F], FP, tag="sel_w")
        nc.vector.select(sel_w[:], mask16[:], expsel[:], neg1[:])
        packed_w = rpool.tile([16, SG_O], FP, tag="packed_w")
        nf2 = rpool.tile([1, 1], U32, tag="nf2")
        nc.gpsimd.sparse_gather(out=packed_w[:], in_=sel_w[:],
                                num_found=nf2[:])
        zp = rpool.tile([16, 1], FP, tag="zp")
        nc.vector.reduce_sum(zp[:], expsel[:], axis=mybir.AxisListType.X)
        zpt = ps_x.tile([1, 16], FP, tag="xpose_ps", name="zp_t")
        nc.tensor.transpose(zpt[:], zp[:], ident_f[:])
        zs = rpool.tile([1, 1], FP, tag="zs")
        nc.vector.reduce_sum(zs[:], zpt[:], axis=mybir.AxisListType.X)
        rz1 = rpool.tile([1, 1], FP, tag="rz1")
        nc.vector.reciprocal(rz1[:], zs[:])
        rzp = ps_x.tile([128, 1], FP, tag="xpose_ps", name="rz_bc")
        nc.tensor.matmul(rzp[:], lhsT=ones_row[:], rhs=rz1[:],
                         start=True, stop=True)
        rzb = rpool.tile([128, 1], FP, tag="rzb")
        nc.vector.tensor_copy(rzb[:], rzp[:])
        wps = ps_x.tile([SG_O, 16], FP, tag="xpose_ps", name="w_t")
        nc.tensor.transpose(wps[:], packed_w[:], ident_f[:])
        w_sb = rpool.tile([SG_O, 16], FP, tag="w_sb")
        nc.vector.tensor_copy(w_sb[:], wps[:])
        w_bnc = dram.tile([SG_O, 16], FP, tag="w_bnc")
        nc.sync.dma_start(w_bnc[:], w_sb[:])
        w_lin = w_bnc[:].rearrange("p f -> (p f)").unsqueeze(1)
        w_cols = []
        for g in range(cfg.G):
            wcf = rpool.tile([128, 1], FP, tag=f"wcf_{g}")
            nc.sync.dma_start(wcf[:], w_lin[ts(g, 128)])
            wcn = rpool.tile([128, 1], FP, tag=f"wcn_{g}")
            nc.vector.tensor_tensor(out=wcn[:], in0=wcf[:], in1=rzb[:],
                                    op=mybir.AluOpType.mult)
            w_cols.append(wcn)

        # --- debug ---
        dbg_sb = rpool.tile([1, 4], FP, tag="dbg")
        nc.vector.tensor_copy(dbg_sb[:, 0:1], nf[:])
        nc.vector.tensor_copy(dbg_sb[:, 1:2], thr_f[:1, :])
        nc.vector.tensor_copy(dbg_sb[:, 2:3], zs[:1, :])
        nc.vector.tensor_copy(dbg_sb[:, 3:4], mxs[:1, :])
        nc.sync.dma_start(dbg_d[:, :], dbg_sb[:])

        # --- FFN, token-chunk outer so y(t) overlaps h(t+1) ---
        CCH = min(512, C)
        aT = [None] * cfg.HK
        for t in range(cfg.NTOK):
            for hc in range(cfg.HK):
                p1 = ps_h1.tile([128, cfg.TOKCH], FP, tag="h1")
                p3 = ps_h3.tile([128, cfg.TOKCH], FP, tag="h3")
                for c in range(cfg.CK):
                    nc.tensor.matmul(
                        p1[:], lhsT=w1sb[:, ts(hc * cfg.CK + c, 128)],
                        rhs=xgT[c][t][:],
                        start=(c == 0), stop=(c == cfg.CK - 1))
                for c in range(cfg.CK):
                    nc.tensor.matmul(
                        p3[:], lhsT=w3sb[:, ts(hc * cfg.CK + c, 128)],
                        rhs=xgT[c][t][:],
                        start=(c == 0), stop=(c == cfg.CK - 1))
                sg = gpool.tile([128, cfg.TOKCH], FP, tag="sigm")
                nc.scalar.activation(sg[:], p1[:],
                                     mybir.ActivationFunctionType.Sigmoid)
                sl = gpool.tile([128, cfg.TOKCH], FP, tag="silu")
                nc.vector.tensor_tensor(out=sl[:], in0=sg[:], in1=p1[:],
                                        op=mybir.AluOpType.mult)
                a_t = apool.tile([128, cfg.TOKCH], BF, tag=f"a_{hc}", bufs=1,
                                 name=f"a_{hc}_{t}")
                nc.vector.tensor_tensor(out=a_t[:], in0=sl[:], in1=p3[:],
                                        op=mybir.AluOpType.mult)
                aT[hc] = a_t
            for col in range(cfg.GPT):
                g = t * cfg.GPT + col
                for cc in range(C // CCH):
                    yp = ps_y.tile([128, CCH], FP, tag="ypsum")
                    for hc in range(cfg.HK):
                        nc.tensor.matmul(yp[:],
                                         lhsT=aT[hc][:, ts(col, 128)],
                                         rhs=w2sb[hc][:, ts(cc, CCH)],
                                         start=(hc == 0),
                                         stop=(hc == cfg.HK - 1))
                    ysb = ypool.tile([128, CCH], FP, tag="ysb")
                    nc.vector.tensor_scalar_mul(ysb[:], yp[:], w_cols[g][:])
                    nc.sync.dma_start(y_d[ts(g, 128), ts(cc, CCH)], ysb[:])

    nc.compile()
    return nc


_CACHE = {}


def _get_program(cfg: Cfg):
    key = (cfg.N, cfg.C, cfg.H, cfg.E)
    if key not in _CACHE:
        _CACHE[key] = build_program(cfg)
    return _CACHE[key]


def make_in_maps(cfg: Cfg, x_flat, Wg, W1, W2, W3):
    """Per-core input dicts (host-side sharding / pre-transposing)."""
    bf = ml_dtypes.bfloat16
    x_flat = np.ascontiguousarray(x_flat, dtype=np.float32)
    # [p, k, e] layout so the gate-weight load is one contiguous DMA
    wgT = np.ascontiguousarray(
        Wg.T.astype(np.float32).reshape(-1, 128, Wg.shape[0]).transpose(1, 0, 2))

    def tile_wT(w):
        # [H, C] -> [128(p), HK*CK*128] with [p, (hc ck f)] = W.T blocks:
        # block (hc, c) at [:, (hc*CK+c)*128:...] = W.T[c*128:(c+1)*128,
        #                                               hc*128:(hc+1)*128]
        wT = w.T.astype(bf)  # [C, H]
        blk = wT.reshape(cfg.CK, 128, cfg.HK, 128)       # [c, p, hc, f]
        return np.ascontiguousarray(
            blk.transpose(1, 2, 0, 3).reshape(128, cfg.HK * cfg.CK * 128))

    iotash = np.zeros((128, 60), dtype=np.float32)
    for r in range(4):
        sh = 12 - 4 * r
        iotash[:, r * 15:(r + 1) * 15] = (
            np.arange(1, 16, dtype=np.float32) * float(1 << sh))

    xbf = x_flat.astype(bf)
    consts = {
        "identbf": np.eye(128, dtype=np.float32).astype(bf),
        "identf": np.eye(16, dtype=np.float32),
        "iotash": iotash,
        "idsf": np.arange(cfg.N, dtype=np.float32).reshape(
            16, cfg.N // 16),
    }
    in_maps = []
    for e in range(NCORES):
        sl = x_flat[e * cfg.TS:(e + 1) * cfg.TS, :]
        in_maps.append({
            "xbf": xbf,
            "xts": np.ascontiguousarray(sl.T),
            "wgT": wgT,
            "w1T": tile_wT(W1[e]),
            "w3T": tile_wT(W3[e]),
            "w2T": np.ascontiguousarray(W2[e].T).astype(bf),
            **consts,
        })
    return in_maps


def combine(cfg: Cfg, results):
    """Host scatter-add of per-expert compact outputs."""
    out = np.zeros((cfg.N, cfg.C), dtype=np.float32)
    for e in range(NCORES):
        ids = np.rint(results[e]["meta"].reshape(-1)).astype(np.int64)
        y = results[e]["y"]
        out[ids] += y
    return out


def kernel(x, Wg, W1, W2, W3):
    cfg = Cfg(N=int(np.prod(x.shape[:-1])), C=x.shape[-1],
              H=W1.shape[1], E=Wg.shape[0])
    nc = _get_program(cfg)
    in_maps = make_in_maps(cfg, np.asarray(x).reshape(-1, cfg.C),
                           np.asarray(Wg), np.asarray(W1), np.asarray(W2),
                           np.asarray(W3))
    res = bass_utils.run_bass_kernel_spmd(nc, in_maps,
                                          core_ids=list(range(NCORES)))
    out = combine(cfg, res.results)
    return out.reshape(x.shape).astype(np.float32)
